# revision 1
# baseline (speedup 1.0000x reference)
"""Distributed GAT (nn_AdjGAT) kernel for 8 TRN2 NeuronCores.

Math: the reference's per-edge softmax logit depends ONLY on the source
node (scores[h,v,k] = attn[h, adj[v,k]]), so
    softmax_k -> w[n_k] / sum_j w[n_j],  w = exp(attn)
    head[h,v] = (sum_k w[h,n_k] * t[h,n_k]) / (sum_k w[h,n_k])
    out = relu(mean_h(head + b[h]))
Each core computes per-node rows [w*t_0..w*t_3 | w_0..w_3] (bf16, padded
to 640 elems = 1280B), AllGathers the row table, then per 128-node chunk:
one dma_gather of 2048 neighbor rows + block-diagonal-ones matmuls that
sum each node's K=16 rows (numerator AND denominator in one pass), then
normalize/relu.  Padding slots point at a hardwired zero row (w=0).
"""

import math
from contextlib import ExitStack

import numpy as np

import concourse.bass as bass
import concourse.bacc as bacc
import concourse.mybir as mybir
from concourse import library_config
from concourse.bass_utils import run_bass_kernel_spmd

F32 = mybir.dt.float32
BF16 = mybir.dt.bfloat16
I16 = mybir.dt.int16
I32 = mybir.dt.int32

V, D, K, O, H = 20000, 256, 16, 128, 4
NCORES = 8


class Cfg:
    def __init__(self, V=V, D=D, K=K, O=O, H=H, ncores=NCORES):
        assert V % ncores == 0
        self.V, self.D, self.K, self.O, self.H, self.ncores = V, D, K, O, H, ncores
        self.VP = V // ncores
        self.ZPAD = 12
        self.VPAD = self.VP + self.ZPAD
        self.VT = self.VPAD * ncores
        self.RW = H * O + 128          # 640 bf16 elems: [s (512) | w (4) | pad]
        self.NCHUNK = math.ceil(self.VP / 128)
        self.VBLK = 512 if self.VP >= 512 else 128 * math.ceil(self.VP / 128)
        self.NBLK = math.ceil(self.VP / self.VBLK)
        self.NJ = self.VBLK // 128
        self.VPF = self.NBLK * self.VBLK
        self.DC = D // 128
        assert D % 128 == 0 and O == 128 and K == 16 and self.DC >= 1
        assert self.VT < 32000


def build_graph(cfg: Cfg, skip_ag=False, only=None, debug=False):
    nc = bacc.Bacc()
    VP, VPAD, VT, RW, H_, O_, DC = (
        cfg.VP, cfg.VPAD, cfg.VT, cfg.RW, cfg.H, cfg.O, cfg.DC)
    NCH, NBLK, VBLK, NJ, VPF = cfg.NCHUNK, cfg.NBLK, cfg.VBLK, cfg.NJ, cfg.VPF
    SW = H_ * O_
    NE = 128 * 16
    NIT = NBLK * H_

    xT = nc.declare_dram_parameter("xT", [cfg.D, VP], F32, isOutput=False)
    Wp = nc.declare_dram_parameter("W", [H_, cfg.D, O_], F32, isOutput=False)
    ap_ = nc.declare_dram_parameter("a", [H_, O_], F32, isOutput=False)
    bp = nc.declare_dram_parameter("b", [H_, O_], F32, isOutput=False)
    eyep = nc.declare_dram_parameter("eye", [128, 128], F32, isOutput=False)
    blkp = nc.declare_dram_parameter("blk", [128, 248], F32, isOutput=False)
    idxp = nc.declare_dram_parameter("idx", [128, NCH * 16], I32, isOutput=False)
    out_ext = nc.declare_dram_parameter("out", [VP, O_], F32, isOutput=True)
    if debug:
        dbg_t0 = nc.declare_dram_parameter("dbg_t0", [128, RW], BF16, isOutput=True)
        dbg_t1 = nc.declare_dram_parameter("dbg_t1", [128, RW], BF16, isOutput=True)
        dbg_gt = nc.declare_dram_parameter("dbg_gt", [128, 16, RW], BF16,
                                           isOutput=True)

    tbl_loc = nc.dram_tensor("tbl_loc", [VPAD, RW], BF16)
    tbl = nc.dram_tensor("tbl", [VT, RW], BF16, addr_space="Shared")

    ctx = ExitStack()
    sb = lambda name, shape, dt: ctx.enter_context(nc.sbuf_tensor(name, shape, dt))
    xT_sb = sb("xT_sb", [128, DC, VPF], BF16)
    W_sb = sb("W_sb", [128, H_, DC, O_], BF16)
    a_sb = sb("a_sb", [128, H_], BF16)
    b_sb = sb("b_sb", [H_, O_], BF16)
    eye_sb = sb("eye_sb", [128, 128], BF16)
    blk_sb = sb("blk_sb", [128, 248], BF16)
    idx_sb = sb("idx_sb", [128, NCH * 16], I32)
    ones1 = sb("ones1", [1, 128], BF16)
    one11 = sb("one11", [1, 1], BF16)
    ones4 = sb("ones4", [H_, O_], BF16)
    zero_sb = sb("zero_sb", [cfg.ZPAD, RW], BF16)
    tB_sb = [sb(f"tB_sb{i}", [128, VBLK], BF16) for i in range(2)]
    sB_sb = [sb(f"sB_sb{i}", [128, VBLK], BF16) for i in range(2)]
    w_row = [sb(f"w_row{i}", [1, VBLK], BF16) for i in range(2)]
    stage = [sb(f"stage{i}", [128, NJ, RW], BF16) for i in range(2)]
    binit_sb = sb("binit_sb", [128, O_], F32)
    gtile = [sb(f"gtile{i}", [128, 16, RW], BF16) for i in range(2)]
    den_sb = [sb(f"den{i}", [128, H_], F32) for i in range(2)]
    rcp_sb = [sb(f"rcp{i}", [128, H_], F32) for i in range(2)]
    acc_sb = [sb(f"acc{i}", [128, O_], F32) for i in range(2)]
    ostage = [sb(f"ostage{i}", [128, O_], F32) for i in range(2)]

    # phase-1 PSUM (freed before phase-2 allocs; full 2KB banks each)
    ph1 = ExitStack()
    psa = lambda name, shape, dt: ph1.enter_context(nc.psum_tensor(name, shape, dt))
    ps_tB = [psa(f"ps_tB{i}", [128, 512], F32) for i in range(2)]
    ps_wbc = [psa(f"ps_wbc{i}", [128, 512], F32) for i in range(2)]
    ps_misc = [psa(f"ps_misc{i}", [128, 512], F32) for i in range(2)]
    ps_C = [psa(f"ps_C{i}", [128, 1024], BF16) for i in range(2)]
    ph1.close()
    ph2 = ExitStack()
    ps_A = [ph2.enter_context(nc.psum_tensor(f"ps_A{i}", [128, 512], F32))
            for i in range(2)]
    ps_B2 = [ph2.enter_context(nc.psum_tensor(f"ps_B2{i}", [128, 512], F32))
             for i in range(2)]
    ph2.close()

    blk_ndma = []
    for bN in range(NBLK):
        lo, hi = bN * VBLK, min(VP, bN * VBLK + VBLK)
        blk_ndma.append((1 if (hi - lo) // 128 else 0) + (1 if (hi - lo) % 128 else 0))
    cum_stdma = np.cumsum(blk_ndma).tolist()

    sctx = ExitStack()
    sem = lambda n: sctx.enter_context(nc.semaphore(n))
    (s_in, s_idx, s_dvi, s_bini, s_bcp, s_tb, s_tbsb, s_attn, s_w, s_sb,
     s_pw, s_pc, s_st, s_zr, s_cc, s_mm, s_ep, s_rel) = [sem(n) for n in (
        "s_in", "s_idx", "s_dvi", "s_bini", "s_bcp", "s_tb", "s_tbsb",
        "s_attn", "s_w", "s_sb", "s_pw", "s_pc", "s_st", "s_zr",
        "s_cc", "s_mm", "s_ep", "s_rel")]
    s_g = [sem("s_g0"), sem("s_g1")]
    s_sd = [sem("s_sd0"), sem("s_sd1")]
    s_o = [sem("s_o0"), sem("s_o1")]
    # cumulative stage-DMA counts per parity (for safe per-parity waits)
    cum_par = []
    c0 = c1 = 0
    for bN in range(NBLK):
        if bN % 2 == 0:
            c0 += blk_ndma[bN]
        else:
            c1 += blk_ndma[bN]
        cum_par.append((c0, c1))
    with nc.Block() as block:
        @block.gpsimd
        def _(g):
            g.dma_start(out=xT_sb[:, :, 0:VP], in_=bass.AP(
                xT, 0, [[VP, 128], [128 * VP, DC], [1, VP]])).then_inc(s_in, 16)
            g.dma_start(out=W_sb[:, :, :, :], in_=bass.AP(
                Wp, 0, [[O_, 128], [cfg.D * O_, H_], [128 * O_, DC], [1, O_]]
            )).then_inc(s_in, 16)
            with nc.allow_non_contiguous_dma(reason="tiny transposed a load"):
                g.dma_start(out=a_sb[:, :], in_=bass.AP(
                    ap_, 0, [[1, 128], [O_, H_]])).then_inc(s_in, 16)
            g.dma_start(out=b_sb[:, :], in_=bass.AP(
                bp, 0, [[O_, H_], [1, O_]])).then_inc(s_in, 16)
            g.dma_start(out=eye_sb[:, :], in_=eyep[:, :]).then_inc(s_in, 16)
            g.dma_start(out=blk_sb[:, :], in_=blkp[:, :]).then_inc(s_in, 16)
            g.wait_ge(s_dvi, 1)
            g.dma_start(out=bass.AP(tbl_loc, VP * RW, [[RW, cfg.ZPAD], [1, RW]]),
                        in_=zero_sb[:, :]).then_inc(s_zr, 16)
            g.wait_ge(s_zr, 16)
            if only != "p2":
                g.wait_ge(s_sd[0], 16 * cum_par[-1][0])
                if cum_par[-1][1]:
                    g.wait_ge(s_sd[1], 16 * cum_par[-1][1])
            if skip_ag:
                g.dma_start(out=bass.AP(tbl, 0, [[RW, VPAD], [1, RW]]),
                            in_=tbl_loc[:, :]).then_inc(s_cc, 16)
                g.wait_ge(s_cc, 16)
            else:
                g.collective_compute(
                    "AllGather", mybir.AluOpType.bypass,
                    replica_groups=[list(range(cfg.ncores))],
                    ins=[tbl_loc[:, :]], outs=[tbl[:, :]],
                ).then_inc(s_cc)
                g.wait_ge(s_cc, 1)
            g.wait_ge(s_idx, 16)
            if debug:
                g.dma_start(out=dbg_t0[:, :], in_=bass.AP(
                    tbl, 0, [[RW, 128], [1, RW]])).then_inc(s_zr, 16)
                g.dma_start(out=dbg_t1[:, :], in_=bass.AP(
                    tbl, VPAD * RW, [[RW, 128], [1, RW]])).then_inc(s_zr, 16)
            for c in (range(NCH) if only not in ("p1", "p2mmn") else []):
                if c >= 2 and only != "p2g":
                    g.wait_ge(s_mm, c - 1)
                for j in range(16):
                    g.indirect_dma_start(
                        out=gtile[c % 2][:, j, :],
                        out_offset=None,
                        in_=tbl[:, :],
                        in_offset=bass.IndirectOffsetOnAxis(
                            ap=idx_sb[:, c * 16 + j:c * 16 + j + 1], axis=0),
                    ).then_inc(s_g[c % 2], 16)

        @block.tensor
        def _(pe):
            pe.wait_ge(s_in, 96)
            pe.wait_ge(s_dvi, 1)
            pe.matmul(ps_misc[0][:, 0:O_], ones4[:, :], b_sb[:, :],
                      start=True, stop=True).then_inc(s_bini, 1)
            for it in (range(NIT) if only != "p2" else []):
                bN, h = divmod(it, H_)
                vlo = bN * VBLK
                if it >= 2:
                    pe.wait_ge(s_tbsb, it - 1)   # ps_tB[it%2] free (ACT copied)
                mm = pe.matmul(ps_tB[it % 2][:, 0:VBLK], W_sb[:, h, 0, :],
                               xT_sb[:, 0, vlo:vlo + VBLK],
                               start=True, stop=(DC == 1))
                for c in range(1, DC):
                    mm = pe.matmul(ps_tB[it % 2][:, 0:VBLK], W_sb[:, h, c, :],
                                   xT_sb[:, c, vlo:vlo + VBLK],
                                   start=False, stop=(c == DC - 1))
                mm.then_inc(s_tb, 1)
                pe.wait_ge(s_tbsb, it + 1)       # tB_sb[it%2] ready
                if it == 0:
                    pe.wait_ge(s_bcp, 1)         # binit copied out of ps_misc[0]
                if it >= 2:
                    pe.wait_ge(s_st, it - 1)     # ps_misc/ps_C[it%2] free (DVE)
                pe.matmul(ps_misc[it % 2][0:1, 0:VBLK], a_sb[:, h:h + 1],
                          tB_sb[it % 2][:, :], start=True, stop=True
                          ).then_inc(s_attn, 1)
                pe.wait_ge(s_w, it + 1)          # w_row[it%2] ready
                if it >= 2:
                    pe.wait_ge(s_sb, it - 1)     # ps_wbc[it%2] free (DVE)
                pe.matmul(ps_wbc[it % 2][:, 0:VBLK], ones1[:, :],
                          w_row[it % 2][:, :], start=True, stop=True)
                for j in range(NJ):
                    e = pe.matmul(ps_misc[it % 2][:, j:j + 1],
                                  w_row[it % 2][0:1, j * 128:(j + 1) * 128],
                                  one11[:, :], start=True, stop=True)
                e.then_inc(s_pw, 1)
                pe.wait_ge(s_sb, it + 1)         # sB_sb[it%2] ready
                for j in range(NJ):
                    e = pe.transpose(ps_C[it % 2][:, j * 128:(j + 1) * 128],
                                     sB_sb[it % 2][:, j * 128:(j + 1) * 128],
                                     eye_sb[:, :])
                e.then_inc(s_pc, 1)
            for c in (range(NCH) if only not in ("p1", "p2g") else []):
                if only != "p2mmn":
                    pe.wait_ge(s_g[c % 2], 256 * (c // 2 + 1))
                if c >= 2 and only not in ("p2mm", "p2mmn"):
                    pe.wait_ge(s_ep, c - 1)      # ps_A/ps_B2[c%2] free (DVE)
                for gi in range(16):
                    pe.matmul(ps_A[c % 2][:, 0:SW],
                              blk_sb[:, 120 - 8 * gi:248 - 8 * gi],
                              gtile[c % 2][:, gi, 0:SW],
                              start=(gi == 0), stop=(gi == 15))
                for gi in range(16):
                    e = pe.matmul(ps_B2[c % 2][:, 0:H_],
                                  blk_sb[:, 120 - 8 * gi:248 - 8 * gi],
                                  gtile[c % 2][:, gi, SW:SW + H_],
                                  start=(gi == 0), stop=(gi == 15))
                e.then_inc(s_mm, 1)

        @block.scalar
        def _(s):
            for it in (range(NIT) if only != "p2" else []):
                s.wait_ge(s_tb, it + 1)
                if it >= 2:
                    s.wait_ge(s_sb, it - 1)      # tB_sb[it%2] free (DVE read)
                s.activation(tB_sb[it % 2][:, :], ps_tB[it % 2][:, 0:VBLK],
                             mybir.ActivationFunctionType.Copy).then_inc(s_tbsb, 1)
                s.wait_ge(s_attn, it + 1)
                if it >= 2:
                    s.wait_ge(s_pw, it - 1)      # w_row[it%2] free (PE read)
                s.activation(w_row[it % 2][:, :], ps_misc[it % 2][0:1, 0:VBLK],
                             mybir.ActivationFunctionType.Exp).then_inc(s_w, 1)
            for c in (range(NCH) if only not in ("p1", "p2mm", "p2g", "p2mmn") else []):
                s.wait_ge(s_ep, c + 1)
                if c >= 2:
                    s.wait_ge(s_o[c % 2], 16 * (c // 2))  # ostage free
                s.activation(ostage[c % 2][:, :], acc_sb[c % 2][:, :],
                             mybir.ActivationFunctionType.Relu,
                             scale=1.0 / H_).then_inc(s_rel, 1)

        @block.vector
        def _(v):
            v.memset(ones1[:, :], 1.0)
            v.memset(one11[:, :], 1.0)
            v.memset(ones4[:, :], 1.0)
            v.memset(stage[0][:, :, SW + H_:RW], 0.0)
            v.memset(stage[1][:, :, SW + H_:RW], 0.0)
            if VPF > VP:
                v.memset(xT_sb[:, :, VP:VPF], 0.0)
            v.memset(zero_sb[:, :], 0.0).then_inc(s_dvi, 1)
            v.wait_ge(s_bini, 1)
            v.tensor_copy(binit_sb[:, :], ps_misc[0][:, 0:O_]).then_inc(s_bcp, 1)
            for it in (range(NIT) if only != "p2" else []):
                bN, h = divmod(it, H_)
                v.wait_ge(s_pw, it + 1)          # ps_wbc ready (+ tB_sb ready)
                if it >= 2:
                    v.wait_ge(s_pc, it - 1)      # sB_sb[it%2] free (PE read)
                v.tensor_tensor(sB_sb[it % 2][:, :], tB_sb[it % 2][:, :],
                                ps_wbc[it % 2][:, 0:VBLK],
                                mybir.AluOpType.mult).then_inc(s_sb, 1)
                v.wait_ge(s_pc, it + 1)          # ps_C ready
                if bN >= 2 and h == 0:
                    v.wait_ge(s_sd[bN % 2], 16 * cum_par[bN - 2][bN % 2])
                v.tensor_copy(
                    stage[bN % 2][:, :, h * O_:(h + 1) * O_],
                    bass.AP(ps_C[it % 2], 0, [[1024, 128], [128, NJ], [1, 128]]))
                v.tensor_copy(
                    stage[bN % 2][:, :, SW + h:SW + h + 1],
                    bass.AP(ps_misc[it % 2], 0, [[512, 128], [1, NJ], [1, 1]])
                ).then_inc(s_st, 1)
            for c in (range(NCH) if only not in ("p1", "p2mm", "p2g", "p2mmn") else []):
                v.wait_ge(s_mm, c + 1)
                if c >= 2:
                    v.wait_ge(s_rel, c - 1)      # acc_sb[c%2] free (ACT read)
                v.tensor_scalar_max(den_sb[c % 2][:, :], ps_B2[c % 2][:, 0:H_],
                                    1e-30)
                v.drain()
                v.reciprocal(rcp_sb[c % 2][:, :], den_sb[c % 2][:, :])
                v.drain()
                v.scalar_tensor_tensor(
                    acc_sb[c % 2][:, :], ps_A[c % 2][:, 0:O_],
                    rcp_sb[c % 2][:, 0:1], binit_sb[:, :],
                    mybir.AluOpType.mult, mybir.AluOpType.add)
                for h in range(1, H_):
                    v.drain()
                    e = v.scalar_tensor_tensor(
                        acc_sb[c % 2][:, :], ps_A[c % 2][:, h * O_:(h + 1) * O_],
                        rcp_sb[c % 2][:, h:h + 1], acc_sb[c % 2][:, :],
                        mybir.AluOpType.mult, mybir.AluOpType.add)
                e.then_inc(s_ep, 1)

        @block.sync
        def _(sy):
            sy.dma_start(out=idx_sb[:, :], in_=idxp[:, :]).then_inc(s_idx, 16)
            for bN in (range(NBLK) if only != "p2" else []):
                sy.wait_ge(s_st, H_ * (bN + 1))
                lo, hi = bN * VBLK, min(VP, bN * VBLK + VBLK)
                full_j, rem = (hi - lo) // 128, (hi - lo) % 128
                if full_j:
                    sy.dma_start(
                        out=bass.AP(tbl_loc, lo * RW,
                                    [[RW, 128], [128 * RW, full_j], [1, RW]]),
                        in_=stage[bN % 2][:, 0:full_j, :]).then_inc(s_sd[bN % 2], 16)
                if rem:
                    sy.dma_start(
                        out=bass.AP(tbl_loc, (lo + full_j * 128) * RW,
                                    [[RW, rem], [1, RW]]),
                        in_=stage[bN % 2][0:rem, full_j, :]).then_inc(s_sd[bN % 2], 16)
            if debug:
                sy.wait_ge(s_g[0], 16)
                sy.dma_start(out=dbg_gt[:, :, :],
                             in_=gtile[0][:, :, :]).then_inc(s_zr, 16)
            for c in (range(NCH) if only not in ("p1", "p2mm", "p2g", "p2mmn") else []):
                sy.wait_ge(s_rel, c + 1)
                lo = c * 128
                rows = min(128, VP - lo)
                sy.dma_start(out=bass.AP(out_ext, lo * O_, [[O_, rows], [1, O_]]),
                             in_=ostage[c % 2][0:rows, :]).then_inc(s_o[c % 2], 16)
            if only not in ("p1", "p2mm", "p2g", "p2mmn"):
                sy.wait_ge(s_o[0], 16 * ((NCH + 1) // 2))
                if NCH > 1:
                    sy.wait_ge(s_o[1], 16 * (NCH // 2))

    sctx.close()
    ctx.close()
    nc.compile()
    return nc


def prep_core_inputs(cfg: Cfg, x, W, a, b, adj_lst, r):
    """Host-side shard/layout prep for core r (index/layout only, no math)."""
    VP, VPAD, NCH = cfg.VP, cfg.VPAD, cfg.NCHUNK
    xs = np.ascontiguousarray(x[r * VP:(r + 1) * VP].T)
    adj = adj_lst[r * VP:(r + 1) * VP]
    rows = np.where(adj == cfg.V, VP,
                    (adj // VP) * VPAD + (adj % VP)).astype(np.int32)
    rows_p = np.full((NCH * 128, cfg.K), VP, np.int32)
    rows_p[:VP] = rows
    # gtile[p, g, :] = tbl[idx[p, c*16+g]]; edge (node 8g+i, k) at p=16i+k
    lin = rows_p.reshape(NCH, 16, 128)           # [c, g, p] (p = i*16 + k)
    idx = np.ascontiguousarray(
        lin.transpose(2, 0, 1).reshape(128, NCH * 16).astype(np.int32))
    blk = np.zeros((128, 248), np.float32)
    blk[np.arange(128), 120 + np.arange(128) // 16] = 1.0
    return {
        "xT": xs.astype(np.float32), "W": np.asarray(W, np.float32),
        "a": np.asarray(a, np.float32), "b": np.asarray(b, np.float32),
        "eye": np.eye(128, dtype=np.float32), "blk": blk, "idx": idx,
    }


_GRAPH_CACHE = {}


def kernel(x, W, a, b, adj_lst, mask_index, _cfg=None, _trace=False):
    cfg = _cfg or Cfg()
    x = np.asarray(x)
    adj_lst = np.asarray(adj_lst)
    assert int(mask_index) == cfg.V
    key = (cfg.V, cfg.ncores)
    if key not in _GRAPH_CACHE:
        _GRAPH_CACHE[key] = build_graph(cfg)
    nc = _GRAPH_CACHE[key]
    in_maps = [prep_core_inputs(cfg, x, W, a, b, adj_lst, r)
               for r in range(cfg.ncores)]
    res = run_bass_kernel_spmd(nc, in_maps, list(range(cfg.ncores)),
                               trace=_trace)
    out = np.concatenate([res.results[r]["out"] for r in range(cfg.ncores)], 0)
    kernel._last_exec_ns = res.exec_time_ns
    return out



# revision 17
# speedup vs baseline: 1.7879x; 1.7879x over previous
"""Distributed GAT (nn_AdjGAT) kernel for 8 TRN2 NeuronCores.

Math: the reference's per-edge softmax logit depends ONLY on the source
node (scores[h,v,k] = attn[h, adj[v,k]]), so
    softmax_k -> w[n_k] / sum_j w[n_j],  w = exp(attn)
    head[h,v] = (sum_k w[h,n_k] * t[h,n_k]) / (sum_k w[h,n_k])
    out = relu(mean_h(head + b[h]))
Each core computes per-node rows [w*t_0..w*t_3 | w_0..w_3] (bf16, padded
to 640 elems = 1280B), AllGathers the row table, then per 128-node chunk:
one dma_gather of 2048 neighbor rows + block-diagonal-ones matmuls that
sum each node's K=16 rows (numerator AND denominator in one pass), then
normalize/relu.  Padding slots point at a hardwired zero row (w=0).
"""

import math
from contextlib import ExitStack

import numpy as np

import concourse.bass as bass
import concourse.bacc as bacc
import concourse.mybir as mybir
from concourse import library_config
from concourse.bass_utils import run_bass_kernel_spmd

F32 = mybir.dt.float32
BF16 = mybir.dt.bfloat16
I16 = mybir.dt.int16
I32 = mybir.dt.int32

V, D, K, O, H = 20000, 256, 16, 128, 4
NCORES = 8


class Cfg:
    def __init__(self, V=V, D=D, K=K, O=O, H=H, ncores=NCORES):
        assert V % ncores == 0
        self.V, self.D, self.K, self.O, self.H, self.ncores = V, D, K, O, H, ncores
        self.VP = V // ncores
        self.ZPAD = 12
        self.VPAD = self.VP + self.ZPAD
        self.VT = self.VPAD * ncores
        self.RW = H * O + 128          # 640 bf16 elems: [s (512) | w (4) | pad]
        # row bytes (2*RW) must be a multiple of 256 for dma_gather
        self.NCHUNK = math.ceil(self.VP / 128)
        self.VBLK = 512 if self.VP >= 512 else 128 * math.ceil(self.VP / 128)
        self.NBLK = math.ceil(self.VP / self.VBLK)
        self.NJ = self.VBLK // 128
        self.VPF = self.NBLK * self.VBLK
        self.DC = D // 128
        assert D % 128 == 0 and O == 128 and K == 16 and self.DC >= 1
        assert self.VT < 32000


def build_graph(cfg: Cfg, skip_ag=False, only=None, debug=False):
    nc = bacc.Bacc()
    VP, VPAD, VT, RW, H_, O_, DC = (
        cfg.VP, cfg.VPAD, cfg.VT, cfg.RW, cfg.H, cfg.O, cfg.DC)
    NCH, NBLK, VBLK, NJ, VPF = cfg.NCHUNK, cfg.NBLK, cfg.VBLK, cfg.NJ, cfg.VPF
    SW = H_ * O_
    NE = 128 * 16
    NIT = NBLK * H_

    xT = nc.declare_dram_parameter("xT", [cfg.D, VP], F32, isOutput=False)
    Wp = nc.declare_dram_parameter("W", [H_, cfg.D, O_], F32, isOutput=False)
    ap_ = nc.declare_dram_parameter("a", [H_, O_], F32, isOutput=False)
    bp = nc.declare_dram_parameter("b", [H_, O_], F32, isOutput=False)
    eyep = nc.declare_dram_parameter("eye", [128, 128], F32, isOutput=False)
    blkp = nc.declare_dram_parameter("blk", [128, 248], F32, isOutput=False)
    idxp = nc.declare_dram_parameter("idx", [128, NCH * 128], I16, isOutput=False)
    out_ext = nc.declare_dram_parameter("out", [VP, O_], F32, isOutput=True)
    if debug:
        dbg_t0 = nc.declare_dram_parameter("dbg_t0", [128, RW], BF16, isOutput=True)
        dbg_t1 = nc.declare_dram_parameter("dbg_t1", [128, RW], BF16, isOutput=True)
        dbg_gt = nc.declare_dram_parameter("dbg_gt", [128, 16, RW], BF16,
                                           isOutput=True)

    tbl_loc = nc.dram_tensor("tbl_loc", [VPAD, RW], BF16)
    tbl = nc.dram_tensor("tbl", [VT, RW], BF16, addr_space="Shared")

    ctx = ExitStack()
    sb = lambda name, shape, dt: ctx.enter_context(nc.sbuf_tensor(name, shape, dt))
    xT_sb = sb("xT_sb", [128, DC, VPF], BF16)
    W_sb = sb("W_sb", [128, H_, DC, O_], BF16)
    a_sb = sb("a_sb", [128, H_], BF16)
    b_sb = sb("b_sb", [H_, O_], BF16)
    eye_sb = sb("eye_sb", [128, 128], BF16)
    blk_sb = sb("blk_sb", [128, 248], BF16)
    idx_sb = sb("idx_sb", [128, NCH * 128], I16)
    ones1 = sb("ones1", [1, 128], BF16)
    one11 = sb("one11", [1, 1], BF16)
    ones4 = sb("ones4", [H_, O_], BF16)
    zero_sb = sb("zero_sb", [cfg.ZPAD, RW], BF16)
    tB_sb = [sb(f"tB_sb{i}", [128, VBLK], BF16) for i in range(2)]
    sB_sb = [sb(f"sB_sb{i}", [128, VBLK], BF16) for i in range(2)]
    w_row = [sb(f"w_row{i}", [1, VBLK], BF16) for i in range(2)]
    stage = [sb(f"stage{i}", [128, NJ, RW], BF16) for i in range(2)]
    binit_sb = sb("binit_sb", [128, O_], F32)
    gtile = [sb(f"gtile{i}", [128, 16, RW], BF16) for i in range(2)]
    den_sb = [sb(f"den{i}", [128, H_], F32) for i in range(2)]
    rcp_sb = [sb(f"rcp{i}", [128, H_], F32) for i in range(2)]
    acc_sb = [sb(f"acc{i}", [128, O_], F32) for i in range(2)]
    ostage = [sb(f"ostage{i}", [128, O_], F32) for i in range(2)]

    # phase-1 PSUM (freed before phase-2 allocs; full 2KB banks each)
    ph1 = ExitStack()
    psa = lambda name, shape, dt: ph1.enter_context(nc.psum_tensor(name, shape, dt))
    ps_tB = [psa(f"ps_tB{i}", [128, 512], F32) for i in range(2)]
    ps_wbc = [psa(f"ps_wbc{i}", [128, 512], F32) for i in range(2)]
    ps_misc = [psa(f"ps_misc{i}", [128, 512], F32) for i in range(2)]
    ps_C = [psa(f"ps_C{i}", [128, 1024], BF16) for i in range(2)]
    ph1.close()
    ph2 = ExitStack()
    ps_A = [ph2.enter_context(nc.psum_tensor(f"ps_A{i}", [128, 512], F32))
            for i in range(2)]
    ps_B2 = [ph2.enter_context(nc.psum_tensor(f"ps_B2{i}", [128, 512], F32))
             for i in range(2)]
    ph2.close()

    blk_ndma = []
    for bN in range(NBLK):
        lo, hi = bN * VBLK, min(VP, bN * VBLK + VBLK)
        blk_ndma.append((1 if (hi - lo) // 128 else 0) + (1 if (hi - lo) % 128 else 0))
    cum_stdma = np.cumsum(blk_ndma).tolist()

    # gather-size probe: chunks use different per-dma_gather index counts
    GPROBE = [1] * 5 + [2] * 5 + [4] * 5 + [8] * 5
    GPROBE = GPROBE[:NCH] + [8] * max(0, NCH - 20)
    cum_gpar = []
    g0 = g1 = 0
    for c in range(NCH):
        if c % 2 == 0:
            g0 += GPROBE[c]
        else:
            g1 += GPROBE[c]
        cum_gpar.append((g0, g1))

    sctx = ExitStack()
    sem = lambda n: sctx.enter_context(nc.semaphore(n))
    (s_in, s_idx, s_dvi, s_bini, s_bcp, s_tb, s_tbsb, s_attn, s_w, s_sb,
     s_pw, s_pc, s_st, s_zr, s_cc, s_mm, s_ep, s_rel) = [sem(n) for n in (
        "s_in", "s_idx", "s_dvi", "s_bini", "s_bcp", "s_tb", "s_tbsb",
        "s_attn", "s_w", "s_sb", "s_pw", "s_pc", "s_st", "s_zr",
        "s_cc", "s_mm", "s_ep", "s_rel")]
    s_g = [sem("s_g0"), sem("s_g1")]
    s_sd = [sem("s_sd0"), sem("s_sd1")]
    s_o = [sem("s_o0"), sem("s_o1")]
    # cumulative stage-DMA counts per parity (for safe per-parity waits)
    cum_par = []
    c0 = c1 = 0
    for bN in range(NBLK):
        if bN % 2 == 0:
            c0 += blk_ndma[bN]
        else:
            c1 += blk_ndma[bN]
        cum_par.append((c0, c1))
    with nc.Block() as block:
        @block.gpsimd
        def _(g):
            g.load_library(library_config.mlp)
            g.dma_start(out=xT_sb[:, :, 0:VP], in_=bass.AP(
                xT, 0, [[VP, 128], [128 * VP, DC], [1, VP]])).then_inc(s_in, 16)
            g.dma_start(out=W_sb[:, :, :, :], in_=bass.AP(
                Wp, 0, [[O_, 128], [cfg.D * O_, H_], [128 * O_, DC], [1, O_]]
            )).then_inc(s_in, 16)
            with nc.allow_non_contiguous_dma(reason="tiny transposed a load"):
                g.dma_start(out=a_sb[:, :], in_=bass.AP(
                    ap_, 0, [[1, 128], [O_, H_]])).then_inc(s_in, 16)
            g.dma_start(out=b_sb[:, :], in_=bass.AP(
                bp, 0, [[O_, H_], [1, O_]])).then_inc(s_in, 16)
            g.dma_start(out=eye_sb[:, :], in_=eyep[:, :]).then_inc(s_in, 16)
            g.dma_start(out=blk_sb[:, :], in_=blkp[:, :]).then_inc(s_in, 16)
            g.wait_ge(s_dvi, 1)
            g.dma_start(out=bass.AP(tbl_loc, VP * RW, [[RW, cfg.ZPAD], [1, RW]]),
                        in_=zero_sb[:, :]).then_inc(s_zr, 16)
            g.wait_ge(s_zr, 16)
            if only != "p2":
                g.wait_ge(s_sd[0], 16 * cum_par[-1][0])
                if cum_par[-1][1]:
                    g.wait_ge(s_sd[1], 16 * cum_par[-1][1])
            if skip_ag:
                g.dma_start(out=bass.AP(tbl, 0, [[RW, VPAD], [1, RW]]),
                            in_=tbl_loc[:, :]).then_inc(s_cc, 16)
                g.wait_ge(s_cc, 16)
            else:
                g.collective_compute(
                    "AllGather", mybir.AluOpType.bypass,
                    replica_groups=[list(range(cfg.ncores))],
                    ins=[tbl_loc[:, :]], outs=[tbl[:, :]],
                ).then_inc(s_cc)
                g.wait_ge(s_cc, 1)
            g.wait_ge(s_idx, 16)
            if debug:
                g.dma_start(out=dbg_t0[:, :], in_=bass.AP(
                    tbl, 0, [[RW, 128], [1, RW]])).then_inc(s_zr, 16)
                g.dma_start(out=dbg_t1[:, :], in_=bass.AP(
                    tbl, VPAD * RW, [[RW, 128], [1, RW]])).then_inc(s_zr, 16)
            for c in (range(NCH) if only not in ("p1", "p2mmn") else []):
                if c >= 2 and only != "p2g":
                    g.wait_ge(s_mm, c - 1)
                G = GPROBE[c]
                NI = 2048 // G
                for m in range(G):
                    g.dma_gather(
                        out_ap=gtile[c % 2][:, m * (16 // G):(m + 1) * (16 // G), :],
                        in_ap=tbl[:, :],
                        idxs_ap=idx_sb[:, c * 128 + m * (128 // G):
                                       c * 128 + (m + 1) * (128 // G)],
                        num_idxs=NI,
                        num_idxs_reg=NI,
                        elem_size=RW,
                        single_packet=False,
                    ).then_inc(s_g[c % 2], 16)

        @block.tensor
        def _(pe):
            pe.wait_ge(s_in, 96)
            pe.wait_ge(s_dvi, 1)
            pe.matmul(ps_misc[0][:, 0:O_], ones4[:, :], b_sb[:, :],
                      start=True, stop=True).then_inc(s_bini, 1)
            for it in (range(NIT) if only != "p2" else []):
                bN, h = divmod(it, H_)
                vlo = bN * VBLK
                if it >= 2:
                    pe.wait_ge(s_tbsb, it - 1)   # ps_tB[it%2] free (ACT copied)
                mm = pe.matmul(ps_tB[it % 2][:, 0:VBLK], W_sb[:, h, 0, :],
                               xT_sb[:, 0, vlo:vlo + VBLK],
                               start=True, stop=(DC == 1))
                for c in range(1, DC):
                    mm = pe.matmul(ps_tB[it % 2][:, 0:VBLK], W_sb[:, h, c, :],
                                   xT_sb[:, c, vlo:vlo + VBLK],
                                   start=False, stop=(c == DC - 1))
                mm.then_inc(s_tb, 1)
                pe.wait_ge(s_tbsb, it + 1)       # tB_sb[it%2] ready
                if it == 0:
                    pe.wait_ge(s_bcp, 1)         # binit copied out of ps_misc[0]
                if it >= 2:
                    pe.wait_ge(s_st, it - 1)     # ps_misc/ps_C[it%2] free (DVE)
                pe.matmul(ps_misc[it % 2][0:1, 0:VBLK], a_sb[:, h:h + 1],
                          tB_sb[it % 2][:, :], start=True, stop=True
                          ).then_inc(s_attn, 1)
                pe.wait_ge(s_w, it + 1)          # w_row[it%2] ready
                if it >= 2:
                    pe.wait_ge(s_sb, it - 1)     # ps_wbc[it%2] free (DVE)
                pe.matmul(ps_wbc[it % 2][:, 0:VBLK], ones1[:, :],
                          w_row[it % 2][:, :], start=True, stop=True)
                for j in range(NJ):
                    e = pe.matmul(ps_misc[it % 2][:, j:j + 1],
                                  w_row[it % 2][0:1, j * 128:(j + 1) * 128],
                                  one11[:, :], start=True, stop=True)
                e.then_inc(s_pw, 1)
                pe.wait_ge(s_sb, it + 1)         # sB_sb[it%2] ready
                for j in range(NJ):
                    e = pe.transpose(ps_C[it % 2][:, j * 128:(j + 1) * 128],
                                     sB_sb[it % 2][:, j * 128:(j + 1) * 128],
                                     eye_sb[:, :])
                e.then_inc(s_pc, 1)
            for c in (range(NCH) if only not in ("p1", "p2g") else []):
                if only != "p2mmn":
                    pe.wait_ge(s_g[c % 2], 16 * cum_gpar[c][c % 2])
                if c >= 2 and only not in ("p2mm", "p2mmn"):
                    pe.wait_ge(s_ep, c - 1)      # ps_A/ps_B2[c%2] free (DVE)
                for gi in range(16):
                    pe.matmul(ps_A[c % 2][:, 0:SW],
                              blk_sb[:, 120 - 8 * gi:248 - 8 * gi],
                              gtile[c % 2][:, gi, 0:SW],
                              start=(gi == 0), stop=(gi == 15))
                for gi in range(16):
                    e = pe.matmul(ps_B2[c % 2][:, 0:H_],
                                  blk_sb[:, 120 - 8 * gi:248 - 8 * gi],
                                  gtile[c % 2][:, gi, SW:SW + H_],
                                  start=(gi == 0), stop=(gi == 15))
                e.then_inc(s_mm, 1)

        @block.scalar
        def _(s):
            for it in (range(NIT) if only != "p2" else []):
                s.wait_ge(s_tb, it + 1)
                if it >= 2:
                    s.wait_ge(s_sb, it - 1)      # tB_sb[it%2] free (DVE read)
                s.activation(tB_sb[it % 2][:, :], ps_tB[it % 2][:, 0:VBLK],
                             mybir.ActivationFunctionType.Copy).then_inc(s_tbsb, 1)
                s.wait_ge(s_attn, it + 1)
                if it >= 2:
                    s.wait_ge(s_pw, it - 1)      # w_row[it%2] free (PE read)
                s.activation(w_row[it % 2][:, :], ps_misc[it % 2][0:1, 0:VBLK],
                             mybir.ActivationFunctionType.Exp).then_inc(s_w, 1)
            for c in (range(NCH) if only not in ("p1", "p2mm", "p2g", "p2mmn") else []):
                s.wait_ge(s_ep, c + 1)
                if c >= 2:
                    s.wait_ge(s_o[c % 2], 16 * (c // 2))  # ostage free
                s.activation(ostage[c % 2][:, :], acc_sb[c % 2][:, :],
                             mybir.ActivationFunctionType.Relu,
                             scale=1.0 / H_).then_inc(s_rel, 1)

        @block.vector
        def _(v):
            v.memset(ones1[:, :], 1.0)
            v.memset(one11[:, :], 1.0)
            v.memset(ones4[:, :], 1.0)
            v.memset(stage[0][:, :, SW + H_:RW], 0.0)
            v.memset(stage[1][:, :, SW + H_:RW], 0.0)
            if VPF > VP:
                v.memset(xT_sb[:, :, VP:VPF], 0.0)
            v.memset(zero_sb[:, :], 0.0).then_inc(s_dvi, 1)
            v.wait_ge(s_bini, 1)
            v.tensor_copy(binit_sb[:, :], ps_misc[0][:, 0:O_]).then_inc(s_bcp, 1)
            for it in (range(NIT) if only != "p2" else []):
                bN, h = divmod(it, H_)
                v.wait_ge(s_pw, it + 1)          # ps_wbc ready (+ tB_sb ready)
                if it >= 2:
                    v.wait_ge(s_pc, it - 1)      # sB_sb[it%2] free (PE read)
                v.tensor_tensor(sB_sb[it % 2][:, :], tB_sb[it % 2][:, :],
                                ps_wbc[it % 2][:, 0:VBLK],
                                mybir.AluOpType.mult).then_inc(s_sb, 1)
                v.wait_ge(s_pc, it + 1)          # ps_C ready
                if bN >= 2 and h == 0:
                    v.wait_ge(s_sd[bN % 2], 16 * cum_par[bN - 2][bN % 2])
                v.tensor_copy(
                    stage[bN % 2][:, :, h * O_:(h + 1) * O_],
                    bass.AP(ps_C[it % 2], 0, [[1024, 128], [128, NJ], [1, 128]]))
                v.tensor_copy(
                    stage[bN % 2][:, :, SW + h:SW + h + 1],
                    bass.AP(ps_misc[it % 2], 0, [[512, 128], [1, NJ], [1, 1]])
                ).then_inc(s_st, 1)
            for c in (range(NCH) if only not in ("p1", "p2mm", "p2g", "p2mmn") else []):
                v.wait_ge(s_mm, c + 1)
                if c >= 2:
                    v.wait_ge(s_rel, c - 1)      # acc_sb[c%2] free (ACT read)
                v.tensor_scalar_max(den_sb[c % 2][:, :], ps_B2[c % 2][:, 0:H_],
                                    1e-30)
                v.drain()
                v.reciprocal(rcp_sb[c % 2][:, :], den_sb[c % 2][:, :])
                v.drain()
                v.scalar_tensor_tensor(
                    acc_sb[c % 2][:, :], ps_A[c % 2][:, 0:O_],
                    rcp_sb[c % 2][:, 0:1], binit_sb[:, :],
                    mybir.AluOpType.mult, mybir.AluOpType.add)
                for h in range(1, H_):
                    v.drain()
                    e = v.scalar_tensor_tensor(
                        acc_sb[c % 2][:, :], ps_A[c % 2][:, h * O_:(h + 1) * O_],
                        rcp_sb[c % 2][:, h:h + 1], acc_sb[c % 2][:, :],
                        mybir.AluOpType.mult, mybir.AluOpType.add)
                e.then_inc(s_ep, 1)

        @block.sync
        def _(sy):
            sy.dma_start(out=idx_sb[:, :], in_=idxp[:, :]).then_inc(s_idx, 16)
            for bN in (range(NBLK) if only != "p2" else []):
                sy.wait_ge(s_st, H_ * (bN + 1))
                lo, hi = bN * VBLK, min(VP, bN * VBLK + VBLK)
                full_j, rem = (hi - lo) // 128, (hi - lo) % 128
                if full_j:
                    sy.dma_start(
                        out=bass.AP(tbl_loc, lo * RW,
                                    [[RW, 128], [128 * RW, full_j], [1, RW]]),
                        in_=stage[bN % 2][:, 0:full_j, :]).then_inc(s_sd[bN % 2], 16)
                if rem:
                    sy.dma_start(
                        out=bass.AP(tbl_loc, (lo + full_j * 128) * RW,
                                    [[RW, rem], [1, RW]]),
                        in_=stage[bN % 2][0:rem, full_j, :]).then_inc(s_sd[bN % 2], 16)
            if debug:
                sy.wait_ge(s_g[0], 16)
                sy.dma_start(out=dbg_gt[:, :, :],
                             in_=gtile[0][:, :, :]).then_inc(s_zr, 16)
            for c in (range(NCH) if only not in ("p1", "p2mm", "p2g", "p2mmn") else []):
                sy.wait_ge(s_rel, c + 1)
                lo = c * 128
                rows = min(128, VP - lo)
                sy.dma_start(out=bass.AP(out_ext, lo * O_, [[O_, rows], [1, O_]]),
                             in_=ostage[c % 2][0:rows, :]).then_inc(s_o[c % 2], 16)
            if only not in ("p1", "p2mm", "p2g", "p2mmn"):
                sy.wait_ge(s_o[0], 16 * ((NCH + 1) // 2))
                if NCH > 1:
                    sy.wait_ge(s_o[1], 16 * (NCH // 2))

    sctx.close()
    ctx.close()
    nc.compile()
    return nc


def prep_core_inputs(cfg: Cfg, x, W, a, b, adj_lst, r):
    """Host-side shard/layout prep for core r (index/layout only, no math)."""
    VP, VPAD, NCH = cfg.VP, cfg.VPAD, cfg.NCHUNK
    xs = np.ascontiguousarray(x[r * VP:(r + 1) * VP].T)
    adj = adj_lst[r * VP:(r + 1) * VP]
    rows = np.where(adj == cfg.V, VP,
                    (adj // VP) * VPAD + (adj % VP)).astype(np.int32)
    rows_p = np.full((NCH * 128, cfg.K), VP, np.int32)
    rows_p[:VP] = rows
    # dma_gather: chunk c gathers edge i = 16*n_local + k (n_local = 0..127)
    # to partition i%128 = (n%8)*16+k, slot i//128 = n//8; index i is read
    # from idxs[i%16, i//16], i.e. idx16[k, c*128+n] = rows_p[c*128+n, k].
    # idx wrapped in 16 partitions, replicated across the 8 Q7 subcores
    # (each subcore reads its own 16-partition stripe)
    idx = np.tile(rows_p.T.astype(np.int16), (8, 1))
    blk = np.zeros((128, 248), np.float32)
    blk[np.arange(128), 120 + np.arange(128) // 16] = 1.0
    return {
        "xT": xs.astype(np.float32), "W": np.asarray(W, np.float32),
        "a": np.asarray(a, np.float32), "b": np.asarray(b, np.float32),
        "eye": np.eye(128, dtype=np.float32), "blk": blk, "idx": idx,
    }


_GRAPH_CACHE = {}


def kernel(x, W, a, b, adj_lst, mask_index, _cfg=None, _trace=False):
    cfg = _cfg or Cfg()
    x = np.asarray(x)
    adj_lst = np.asarray(adj_lst)
    assert int(mask_index) == cfg.V
    key = (cfg.V, cfg.ncores)
    if key not in _GRAPH_CACHE:
        _GRAPH_CACHE[key] = build_graph(cfg)
    nc = _GRAPH_CACHE[key]
    in_maps = [prep_core_inputs(cfg, x, W, a, b, adj_lst, r)
               for r in range(cfg.ncores)]
    res = run_bass_kernel_spmd(nc, in_maps, list(range(cfg.ncores)),
                               trace=_trace)
    out = np.concatenate([res.results[r]["out"] for r in range(cfg.ncores)], 0)
    kernel._last_exec_ns = res.exec_time_ns
    return out

